# revision 3
# baseline (speedup 1.0000x reference)
"""Distributed Trainium2 kernel for nn_Attention_16947940950479.

Reference computation (B=4, S=2048, F=1024, DK=1024):
    q = x @ Wq.T + bq ; k = x @ Wk.T + bk ; v = x @ Wv.T + bv
    scores = (q @ k.T) / sqrt(DK)
    attn = softmax(scores, axis=-2)        # over the QUERY axis
    ctx = attn @ v
    out = ctx @ Wo.T + bo

Sharding (8 NeuronCores): core c = 2*b + h owns batch b, query-half h
(1024 queries). Because the softmax normalizes over queries, scores are
kept transposed [key, query]; the softmax sum is fused into the ScalarE
exp (accum_out) and the only cross-core communication is an AllReduce of
the per-key denominators within each pair ([[0,1],[2,3],[4,5],[6,7]]),
chunked [8,4,4] k-tiles so its latency hides under compute (the middle
chunk also keeps the ncfw collective firmware warm).

Algebraic restructure (all exact):
  - Host prefuses the weights: Wqk = Wq.T@Wk, Wov = Wo@Wv, and the bias
    vectors Wq.T@bk and Wo@bv. Per-key and global score-offset terms
    cancel in the query-axis softmax and are dropped. The per-query
    score offset cq[q] = x_q . (Wq.T@bk) is an O(S*F) host dot product
    (0.02% of the device FLOPs) and is shipped as a tiny input.
  - Device pipeline per core:
      qk^T  = Wqk-contraction of xq^T           (f x q)
      s^T   = xk^T-contraction of qk^T + 1*cq   (k x q)
      p     = exp(s/32); denominators via exp accum + pair AllReduce
      attn  = p * (1/den)  per key (partition scalar)
      xp    = x-contraction of attn             (f x q)
      out^T = Wov-contraction of xp + (Wo@bv)*P + bo,  P[q] = sum_k attn
    This removes the Q/K/V projections and the duplicated K/V work
    entirely.
  - Every K=1 (broadcast-restore) and M=1 (partition-sum) matmul costs a
    full N=512 cycles on the PE, so they are packed 4-at-a-time onto
    disjoint 32-row / 32-col PE subarray groups via tile_position, where
    they execute concurrently:
      * the +cq restores that close each score chain run as 4 concurrent
        K=1 matmuls (row groups 0/32/64/96; cq staged at those SBUF
        partitions),
      * P[q] = sum_k attn is built as 4 concurrent col-tiled M=1 chains
        (4 PSUM banks) + one mask-matmul that both combines the partials
        and broadcasts P to partitions {0,32,64,96},
      * the (Wo@bv)*P restores close out-chains 4-at-a-time the same way.
  - Score/ctx-equivalent chains use open PSUM accumulation groups split
    in halves so independent matmul work covers the DMA ramp-in and the
    final AllReduce's latency.

All matmuls bf16 with f32 PSUM accumulation (fp8 was measured: ~216us
but 6-7% error — quantization noise does not average out in random-sign
contractions, so every fp8 operand leaks its full per-element error into
the output). The host pre-transposes/pre-casts all operands so the
device does no transposes or dtype conversions.
"""

import numpy as np
import ml_dtypes

import concourse.bass as bass
import concourse.mybir as mybir
from concourse import bacc, tile
from concourse.bass_utils import run_bass_kernel_spmd
from concourse.tile_rust import add_dep_helper

B, S, F, DK = 4, 2048, 1024, 1024
N_CORES = 8
SH = S // 2            # queries per core
NQB = SH // 512        # q blocks of 512
NKB = S // 512         # key blocks of 512 (K projection)
NKT = S // 128         # key tiles of 128
NFT = F // 128         # f tiles (contraction of projections)
NDT = DK // 128        # d tiles
SCALE = 1.0 / float(np.sqrt(DK))
BF16 = mybir.dt.bfloat16
F32 = mybir.dt.float32
BF = ml_dtypes.bfloat16

REPLICA_GROUPS = [[0, 1], [2, 3], [4, 5], [6, 7]]

_COMPILED = None
LAST_RESULTS = None


def _build():
    nc = bacc.Bacc(
        "TRN2", target_bir_lowering=False, debug=False, num_devices=N_CORES
    )
    xqT = nc.dram_tensor("xqT", [F, SH], BF16, kind="ExternalInput").ap()
    xkT = nc.dram_tensor("xkT", [F, S], BF16, kind="ExternalInput").ap()
    wqk = nc.dram_tensor("wqk", [F, F], BF16, kind="ExternalInput").ap()
    wovT = nc.dram_tensor("wovT", [F, F], BF16, kind="ExternalInput").ap()
    cq4 = nc.dram_tensor("cq4", [4, SH], BF16, kind="ExternalInput").ap()
    wobv4 = nc.dram_tensor("wobv4", [4, F], BF16, kind="ExternalInput").ap()
    mask4 = nc.dram_tensor("mask4", [128, 128], BF16, kind="ExternalInput").ap()
    bor = nc.dram_tensor("bor", [128, NFT], F32, kind="ExternalInput").ap()
    xkN = nc.dram_tensor("xkN", [S, F], BF16, kind="ExternalInput").ap()
    outT = nc.dram_tensor("outT", [F, SH], F32, kind="ExternalOutput").ap()

    with tile.TileContext(nc) as tc:
        with (
            tc.tile_pool(name="smalls", bufs=1) as smalls,
            tc.tile_pool(name="qkv", bufs=1) as qkv,
            tc.tile_pool(name="psum", bufs=8, space="PSUM") as psum,
            tc.tile_pool(name="dram", bufs=1, space="DRAM") as dram,
        ):
            cq4_t = smalls.tile([128, SH], BF16, name="cq4_t")
            wobv4_t = smalls.tile([128, F], BF16, name="wobv4_t")
            mask4_t = smalls.tile([128, 128], BF16, name="mask4_t")
            ones4_t = smalls.tile([128, 128], BF16, name="ones4_t")
            nc.vector.memset(ones4_t[:], 1.0)
            bo_t = smalls.tile([128, NFT], F32, name="bo_t")
            onec_t = smalls.tile([128, 1], BF16, name="onec_t")
            nc.vector.memset(onec_t[:], 1.0)
            stageP = smalls.tile([128, SH], BF16, name="stageP")
            nc.vector.memset(stageP[:], 0.0)
            P4_sb = smalls.tile([128, SH], BF16, name="P4_sb")
            dacc = smalls.tile([128, 2 * NKT], F32, name="dacc")
            den = smalls.tile([128, NKT], F32, name="den")
            deng = smalls.tile([128, NKT], F32, name="deng")
            inv = smalls.tile([128, NKT], F32, name="inv")
            # ScalarE exp-table warm-up: the first ACTIVATE pays the
            # ~2.7us ACT_TABLE_LOAD; issuing a dummy exp at t=0 hides it
            # under the DMA ramp instead of the first score chunk.
            warm_t = smalls.tile([1, 8], F32, name="warm_t")
            warm2_t = smalls.tile([1, 8], F32, name="warm2_t")
            nc.vector.memset(warm_t[:], 0.0)
            nc.scalar.activation(
                warm2_t[:], warm_t[:], mybir.ActivationFunctionType.Exp
            )
            qkT = [qkv.tile([128, SH], BF16, name=f"qkT{i}") for i in range(NFT)]
            xk_t = [qkv.tile([128, S], BF16, name=f"xk{i}") for i in range(NFT)]
            xkN_t = [qkv.tile([128, F], BF16, name=f"xkN{i}") for i in range(NKT)]

            with tc.tile_pool(name="ph1", bufs=1) as ph1:
                xq_t = [ph1.tile([128, SH], BF16, name=f"xq{i}") for i in range(NFT)]
                wk_t = [ph1.tile([128, F], BF16, name=f"wk{i}") for i in range(NFT)]
                # DMAs in consumption order: Q operands (split in two f
                # halves to let the PE start after only 2MB has landed),
                # then K operands, then V weights.
                for half in range(2):
                    for i in range(half * NFT // 2, (half + 1) * NFT // 2):
                        r = slice(i * 128, (i + 1) * 128)
                        nc.sync.dma_start(wk_t[i][:], wqk[r, :])
                        nc.sync.dma_start(xq_t[i][:], xqT[r, :])
                for j in range(4):
                    nc.sync.dma_start(
                        cq4_t[32 * j : 32 * j + 1, :], cq4[j : j + 1, :]
                    )
                    nc.sync.dma_start(
                        wobv4_t[32 * j : 32 * j + 1, :], wobv4[j : j + 1, :]
                    )
                nc.sync.dma_start(mask4_t[:], mask4)
                nc.sync.dma_start(bo_t[:], bor)
                for i in range(NFT):
                    r = slice(i * 128, (i + 1) * 128)
                    nc.sync.dma_start(xk_t[i][:], xkT[r, :])
                for i in range(NKT):
                    r = slice(i * 128, (i + 1) * 128)
                    nc.sync.dma_start(xkN_t[i][:], xkN[r, :])

                # Fused Q/K: the host precomputes Wqk = Wq.T @ Wk, so
                # qk^T[f, q] = sum_f1 Wqk[f1, f] * xq^T[f1, q] directly from
                # the input activations (no Q or K projection on device).
                # Chains split into f1-halves (A: 0..3, B: 4..7) in groups
                # of 8 open PSUM accumulations so the A parts only need the
                # first half of the DMAs.
                qchains = [(fi, qb) for fi in range(NFT) for qb in range(NQB)]
                for grp in range(0, len(qchains), 8):
                    group = qchains[grp : grp + 8]
                    # First group starts after only 2 of 8 operand-tile DMA
                    # pairs (1MB) so the PE ramps in earlier.
                    asplit = 2 if grp == 0 else NFT // 2
                    qps = {}
                    for fi, qb in group:
                        fsl = slice(fi * 128, (fi + 1) * 128)
                        qsl = slice(qb * 512, (qb + 1) * 512)
                        ps = psum.tile([128, 512], F32, name="ps", tag="ps")
                        qps[(fi, qb)] = ps
                        for f1 in range(asplit):
                            nc.tensor.matmul(
                                ps[:], wk_t[f1][:, fsl], xq_t[f1][:, qsl],
                                start=(f1 == 0), stop=False,
                            )
                    for fi, qb in group:
                        fsl = slice(fi * 128, (fi + 1) * 128)
                        qsl = slice(qb * 512, (qb + 1) * 512)
                        ps = qps[(fi, qb)]
                        for f1 in range(asplit, NFT):
                            nc.tensor.matmul(
                                ps[:], wk_t[f1][:, fsl], xq_t[f1][:, qsl],
                                start=False, stop=(f1 == NFT - 1),
                            )
                        nc.vector.tensor_copy(qkT[fi][:, qsl], ps[:])

            with tc.tile_pool(name="ph2", bufs=1) as ph2:
                p_t = [ph2.tile([128, SH], BF16, name=f"p{i}") for i in range(NKT)]
                wov_t = [ph2.tile([128, F], BF16, name=f"wov{i}") for i in range(NFT)]
                xp_t = [ph2.tile([128, SH], BF16, name=f"xp{i}") for i in range(NFT)]
                for i in range(NFT):
                    nc.sync.dma_start(wov_t[i][:], wovT[i * 128 : (i + 1) * 128, :])

                # scores^T[k, q] -> exp(scale*.) -> p (bf16) + per-key rowsums.
                # The key axis is processed in 3 chunks of [8,4,4] k-tiles;
                # each chunk's denominator AllReduce is issued as soon as the
                # chunk's scores are done, so chunk 0's collective hides
                # under chunk 1's score matmuls and the last chunk's
                # collective hides under the ctx matmuls on earlier chunks.
                # The +cq restores that close each (ki, qb) chain are K=1
                # matmuls; they are deferred and issued 4-at-a-time on
                # disjoint PE row groups (tile_position) so 4 of them cost
                # ~512 cycles instead of 4*512.
                CH_BOUNDS = [0, 8, 12, 16]   # k-tile chunk boundaries
                NCH = len(CH_BOUNDS) - 1  # k-tiles per chunk
                prev_readback = None
                cc_ins = [
                    dram.tile([128, CH_BOUNDS[c + 1] - CH_BOUNDS[c]], F32,
                              name=f"cc_in{c}")
                    for c in range(NCH)
                ]
                cc_outs = [
                    dram.tile([128, CH_BOUNDS[c + 1] - CH_BOUNDS[c]], F32,
                              name=f"cc_out{c}")
                    for c in range(NCH)
                ]
                for ch in range(NCH):
                    for ki0 in range(CH_BOUNDS[ch], CH_BOUNDS[ch + 1], 2):
                        # one restore-group = 2 k-tiles x 2 q-blocks
                        pss = {}
                        for ki in (ki0, ki0 + 1):
                            ksl = slice(ki * 128, (ki + 1) * 128)
                            for qb in range(NQB):
                                pss[(ki, qb)] = psum.tile(
                                    [128, 512], F32, name="ps", tag="ps"
                                )
                            # stationary-major: both q-block chains consume
                            # the same xk stationary tile back-to-back,
                            # halving the LDWEIGHTS traffic.
                            for fi in range(NFT):
                                for qb in range(NQB):
                                    qsl = slice(qb * 512, (qb + 1) * 512)
                                    nc.tensor.matmul(
                                        pss[(ki, qb)][:], xk_t[fi][:, ksl],
                                        qkT[fi][:, qsl],
                                        start=(fi == 0), stop=False,
                                    )
                        # 4 concurrent K=1 +cq restores on row groups
                        # 0/32/64/96 (cq staged at those partitions).
                        for ki in (ki0, ki0 + 1):
                            for qb in range(NQB):
                                j = (ki - ki0) * 2 + qb
                                qsl = slice(qb * 512, (qb + 1) * 512)
                                nc.tensor.matmul(
                                    pss[(ki, qb)][:],
                                    ones4_t[32 * j : 32 * j + 1, :],
                                    cq4_t[32 * j : 32 * j + 1, qsl],
                                    start=False, stop=True,
                                    tile_position=(32 * j, 0),
                                )
                        for ki in (ki0, ki0 + 1):
                            for qb in range(NQB):
                                qsl = slice(qb * 512, (qb + 1) * 512)
                                jj = qb * NKT + ki
                                nc.scalar.activation(
                                    p_t[ki][:, qsl], pss[(ki, qb)][:],
                                    mybir.ActivationFunctionType.Exp,
                                    scale=SCALE,
                                    accum_out=dacc[:, jj : jj + 1],
                                )
                    # local chunk denominators -> pair AllReduce -> 1/x
                    c0, c1 = CH_BOUNDS[ch], CH_BOUNDS[ch + 1]
                    csl = slice(c0, c1)
                    nc.vector.tensor_add(
                        den[:, csl], dacc[:, c0:c1], dacc[:, NKT + c0 : NKT + c1]
                    )
                    cin_dma = nc.gpsimd.dma_start(cc_ins[ch][:], den[:, csl])
                    if ch > 0 and prev_readback is not None:
                        # Keep the gpsimd stream in dataflow order: chunk
                        # ch's bounce write must not be scheduled ahead of
                        # chunk ch-1's result readback, else the readback
                        # (and the dependent attn scaling) stalls behind
                        # chunk ch's exp tail.
                        add_dep_helper(
                            cin_dma.ins, prev_readback.ins, False,
                            "AR bounce order: readback before next chunk in",
                        )
                    nc.gpsimd.collective_compute(
                        "AllReduce",
                        mybir.AluOpType.add,
                        replica_groups=REPLICA_GROUPS,
                        ins=[cc_ins[ch].opt()],
                        outs=[cc_outs[ch].opt()],
                    )
                    prev_readback = nc.gpsimd.dma_start(deng[:, csl], cc_outs[ch][:])
                    nc.vector.reciprocal(inv[:, csl], deng[:, csl])
                    # attn^T = p * inv[k]  (per-partition scalar, in place)
                    for ki in range(CH_BOUNDS[ch], CH_BOUNDS[ch + 1]):
                        nc.vector.tensor_scalar_mul(
                            p_t[ki][:], p_t[ki][:], inv[:, ki : ki + 1]
                        )

                # Associativity rewrite of the V side: instead of
                # projecting V for all 2048 keys and ctx = attn^T-contracted
                # V, compute xp[f, q] = sum_k x[k, f] attn[k, q] (per-query,
                # no duplicated work) and apply Wv afterwards:
                # ctx[d, q] = sum_f Wv[d, f] xp[f, q] + bv[d] * P[q] with
                # P[q] = sum_k attn[k, q]. Chains split by k-chunk (A:
                # tiles before the last AllReduce chunk, B: rest) in groups
                # of 8 open PSUM accumulations so the A parts execute while
                # the last denominator AllReduce is in flight.
                cchains = [(fi, qb) for fi in range(NFT) for qb in range(NQB)]
                for grp in range(0, len(cchains), 8):
                    group = cchains[grp : grp + 8]
                    cps = {
                        c: psum.tile([128, 512], F32, name="ps", tag="ps")
                        for c in group
                    }
                    # stationary-major: for each k-tile, the q-block pair of
                    # every fi chain reuses the same xkN stationary tile.
                    for ki in range(CH_BOUNDS[-2]):
                        for fi, qb in group:
                            fsl = slice(fi * 128, (fi + 1) * 128)
                            qsl = slice(qb * 512, (qb + 1) * 512)
                            nc.tensor.matmul(
                                cps[(fi, qb)][:], xkN_t[ki][:, fsl],
                                p_t[ki][:, qsl],
                                start=(ki == 0), stop=False,
                            )
                    for ki in range(CH_BOUNDS[-2], NKT):
                        for fi, qb in group:
                            fsl = slice(fi * 128, (fi + 1) * 128)
                            qsl = slice(qb * 512, (qb + 1) * 512)
                            nc.tensor.matmul(
                                cps[(fi, qb)][:], xkN_t[ki][:, fsl],
                                p_t[ki][:, qsl],
                                start=False, stop=(ki == NKT - 1),
                            )
                    for fi, qb in group:
                        fsl = slice(fi * 128, (fi + 1) * 128)
                        qsl = slice(qb * 512, (qb + 1) * 512)
                        nc.vector.tensor_copy(xp_t[fi][:, qsl], cps[(fi, qb)][:])

                # P[q] = sum_k attn[k, q]: 4 concurrent col-tiled M=1
                # chains (4 k-tiles each, own PSUM banks, output partition
                # 32j), then one mask-matmul that sums the 4 partials and
                # broadcasts P to partitions {0,32,64,96} for the packed
                # out-chain restores.
                for qb in range(NQB):
                    qsl = slice(qb * 512, (qb + 1) * 512)
                    psP = [
                        psum.tile([128, 512], F32, name="psp", tag="ps")
                        for _ in range(4)
                    ]
                    for j in range(4):
                        for t in range(4):
                            ki = 4 * j + t
                            nc.tensor.matmul(
                                psP[j][32 * j : 32 * j + 1, :],
                                onec_t[:, 0:1], p_t[ki][:, qsl],
                                start=(t == 0), stop=(t == 3),
                                tile_position=(0, 32 * j),
                            )
                    for j in range(4):
                        nc.vector.tensor_copy(
                            stageP[32 * j : 32 * j + 1, qsl],
                            psP[j][32 * j : 32 * j + 1, :],
                        )
                    psC = psum.tile([128, 512], F32, name="psc", tag="ps")
                    nc.tensor.matmul(
                        psC[:], mask4_t[:], stageP[:, qsl],
                        start=True, stop=True,
                    )
                    nc.vector.tensor_copy(P4_sb[0:97, qsl], psC[0:97, :])

                # out^T[f', q] = sum_f (Wo@Wv)[f', f] xp[f, q]
                #                + (Wo@bv)[f'] P[q] + bo[f']
                # Chains in groups of 4 so the K=1 (Wo@bv)*P restores can
                # close 4 chains concurrently on disjoint PE row groups.
                ochains = [(fi, qb) for fi in range(NFT) for qb in range(NQB)]
                for grp in range(0, len(ochains), 4):
                    group = ochains[grp : grp + 4]
                    ops = {}
                    for fi, qb in group:
                        fsl = slice(fi * 128, (fi + 1) * 128)
                        qsl = slice(qb * 512, (qb + 1) * 512)
                        ps = psum.tile([128, 512], F32, name="ps", tag="ps")
                        ops[(fi, qb)] = ps
                        for fj in range(NFT):
                            nc.tensor.matmul(
                                ps[:], wov_t[fj][:, fsl], xp_t[fj][:, qsl],
                                start=(fj == 0), stop=False,
                            )
                    for j, (fi, qb) in enumerate(group):
                        fsl = slice(fi * 128, (fi + 1) * 128)
                        qsl = slice(qb * 512, (qb + 1) * 512)
                        nc.tensor.matmul(
                            ops[(fi, qb)][:],
                            wobv4_t[32 * j : 32 * j + 1, fsl],
                            P4_sb[32 * j : 32 * j + 1, qsl],
                            start=False, stop=True,
                            tile_position=(32 * j, 0),
                        )
                    for fi, qb in group:
                        fsl = slice(fi * 128, (fi + 1) * 128)
                        qsl = slice(qb * 512, (qb + 1) * 512)
                        ot = ph2.tile([128, 512], F32, name="ost", tag="ost", bufs=3)
                        nc.vector.tensor_scalar_add(
                            ot[:], ops[(fi, qb)][:], bo_t[:, fi : fi + 1]
                        )
                        nc.sync.dma_start(outT[fsl, qsl], ot[:])

    nc.compile()
    return nc


def _get_compiled():
    global _COMPILED
    if _COMPILED is None:
        _COMPILED = _build()
    return _COMPILED


def kernel(x, Wq, bq, Wk, bk, Wv, bv, Wo, bo):
    global LAST_RESULTS
    nc = _get_compiled()

    x = np.asarray(x, dtype=np.float32)
    Wqf = np.asarray(Wq, np.float32)
    Wkf = np.asarray(Wk, np.float32)
    Wvf = np.asarray(Wv, np.float32)
    Wof = np.asarray(Wo, np.float32)
    wqk = np.ascontiguousarray(Wqf.T @ Wkf).astype(BF)
    wovT = np.ascontiguousarray((Wof @ Wvf).T).astype(BF)
    wqbk = Wqf.T @ np.asarray(bk, np.float32)          # [F]
    wobv = Wof @ np.asarray(bv, np.float32)            # [F]
    wobv4 = np.ascontiguousarray(
        np.broadcast_to(wobv[None, :], (4, F))
    ).astype(BF)
    bor = np.ascontiguousarray(np.asarray(bo, np.float32).reshape(NFT, 128).T)
    m4 = np.zeros((128, 128), np.float32)
    for j in range(4):
        for m in range(4):
            m4[32 * j, 32 * m] = 1.0
    mask4 = m4.astype(BF)

    shared = {
        "wqk": wqk, "wovT": wovT, "wobv4": wobv4, "mask4": mask4, "bor": bor,
    }
    xkT_b = [np.ascontiguousarray(x[b].T).astype(BF) for b in range(B)]
    xkN_b = [np.ascontiguousarray(x[b]).astype(BF) for b in range(B)]
    cq_b = [x[b] @ wqbk for b in range(B)]             # [S] per batch
    in_maps = []
    for c in range(N_CORES):
        b, h = c // 2, c % 2
        xqT_c = np.ascontiguousarray(x[b, h * SH : (h + 1) * SH, :].T).astype(BF)
        cq_c = np.ascontiguousarray(
            np.broadcast_to(cq_b[b][None, h * SH : (h + 1) * SH], (4, SH))
        ).astype(BF)
        in_maps.append(
            {"xqT": xqT_c, "xkT": xkT_b[b], "xkN": xkN_b[b], "cq4": cq_c,
             **shared}
        )

    res = run_bass_kernel_spmd(nc, in_maps, list(range(N_CORES)))
    LAST_RESULTS = res

    out = np.empty((B, S, F), np.float32)
    for c in range(N_CORES):
        b, h = c // 2, c % 2
        out[b, h * SH : (h + 1) * SH, :] = res.results[c]["outT"].T
    return out


# revision 8
# speedup vs baseline: 1.0316x; 1.0316x over previous
"""Distributed Trainium2 kernel for nn_Attention_16947940950479.

Reference computation (B=4, S=2048, F=1024, DK=1024):
    q = x @ Wq.T + bq ; k = x @ Wk.T + bk ; v = x @ Wv.T + bv
    scores = (q @ k.T) / sqrt(DK)
    attn = softmax(scores, axis=-2)        # over the QUERY axis
    ctx = attn @ v
    out = ctx @ Wo.T + bo

Sharding (8 NeuronCores): core c = 2*b + h owns batch b, query-half h
(1024 queries). Because the softmax normalizes over queries, scores are
kept transposed [key, query]; the softmax sum is fused into the ScalarE
exp (accum_out) and the only cross-core communication is an AllReduce of
the per-key denominators within each pair ([[0,1],[2,3],[4,5],[6,7]]),
chunked [8,8] k-tiles so its latency hides under compute; a dummy
AllReduce at t~0 absorbs the ncfw firmware cold-start (~29us measured)
so the real ones run at their warm ~6us latency.

Algebraic restructure (all exact):
  - Host prefuses the weights: Wqk = Wq.T@Wk, Wov = Wo@Wv, and the bias
    vectors Wq.T@bk and Wo@bv. Per-key and global score-offset terms
    cancel in the query-axis softmax and are dropped. The per-query
    score offset cq[q] = x_q . (Wq.T@bk) is an O(S*F) host dot product
    (0.02% of the device FLOPs) and is shipped as a tiny input.
  - Device pipeline per core:
      qk^T  = Wqk-contraction of xq^T           (f x q)
      s^T   = xk^T-contraction of qk^T + 1*cq   (k x q)
      p     = exp(s/32); denominators via exp accum + pair AllReduce
      attn  = p * (1/den)  per key (partition scalar)
      xp    = x-contraction of attn             (f x q)
      out^T = Wov-contraction of xp + (Wo@bv)*P + bo,  P[q] = sum_k attn
    This removes the Q/K/V projections and the duplicated K/V work
    entirely.
  - Every K=1 (broadcast-restore) and M=1 (partition-sum) matmul costs a
    full N=512 cycles on the PE, so they are packed 4-at-a-time onto
    disjoint 32-row / 32-col PE subarray groups via tile_position, where
    they execute concurrently:
      * the +cq restores that close each score chain run as 4 concurrent
        K=1 matmuls (row groups 0/32/64/96; cq staged at those SBUF
        partitions),
      * P[q] = sum_k attn is built as 4 concurrent col-tiled M=1 chains
        (4 PSUM banks) + one mask-matmul that both combines the partials
        and broadcasts P to partitions {0,32,64,96},
      * the (Wo@bv)*P restores close out-chains 4-at-a-time the same way.
  - Score/ctx-equivalent chains use open PSUM accumulation groups split
    in halves so independent matmul work covers the DMA ramp-in and the
    final AllReduce's latency.

All matmuls bf16 with f32 PSUM accumulation (fp8 was measured: ~216us
but 6-7% error — quantization noise does not average out in random-sign
contractions, so every fp8 operand leaks its full per-element error into
the output). The host pre-transposes/pre-casts all operands so the
device does no transposes or dtype conversions.
"""

import numpy as np
import ml_dtypes

import concourse.bass as bass
import concourse.mybir as mybir
from concourse import bacc, tile
from concourse.bass_utils import run_bass_kernel_spmd
from concourse.tile_rust import add_dep_helper

B, S, F, DK = 4, 2048, 1024, 1024
N_CORES = 8
SH = S // 2            # queries per core
NQB = SH // 512        # q blocks of 512
NKB = S // 512         # key blocks of 512 (K projection)
NKT = S // 128         # key tiles of 128
NFT = F // 128         # f tiles (contraction of projections)
NDT = DK // 128        # d tiles
SCALE = 1.0 / float(np.sqrt(DK))
BF16 = mybir.dt.bfloat16
F32 = mybir.dt.float32
BF = ml_dtypes.bfloat16

REPLICA_GROUPS = [[0, 1], [2, 3], [4, 5], [6, 7]]

_COMPILED = None
LAST_RESULTS = None


def _build():
    nc = bacc.Bacc(
        "TRN2", target_bir_lowering=False, debug=False, num_devices=N_CORES
    )
    xqT = nc.dram_tensor("xqT", [F, SH], BF16, kind="ExternalInput").ap()
    xkT = nc.dram_tensor("xkT", [F, S], BF16, kind="ExternalInput").ap()
    wqk = nc.dram_tensor("wqk", [F, F], BF16, kind="ExternalInput").ap()
    wovT = nc.dram_tensor("wovT", [F, F], BF16, kind="ExternalInput").ap()
    cq4 = nc.dram_tensor("cq4", [4, SH], BF16, kind="ExternalInput").ap()
    wobv4 = nc.dram_tensor("wobv4", [4, F], BF16, kind="ExternalInput").ap()
    mask4 = nc.dram_tensor("mask4", [128, 128], BF16, kind="ExternalInput").ap()
    bor = nc.dram_tensor("bor", [128, NFT], F32, kind="ExternalInput").ap()
    xkN = nc.dram_tensor("xkN", [S, F], BF16, kind="ExternalInput").ap()
    outT = nc.dram_tensor("outT", [F, SH], F32, kind="ExternalOutput").ap()

    with tile.TileContext(nc) as tc:
        with (
            tc.tile_pool(name="smalls", bufs=1) as smalls,
            tc.tile_pool(name="qkv", bufs=1) as qkv,
            tc.tile_pool(name="psum", bufs=8, space="PSUM") as psum,
            tc.tile_pool(name="dram", bufs=1, space="DRAM") as dram,
        ):
            cq4_t = smalls.tile([128, SH], BF16, name="cq4_t")
            wobv4_t = smalls.tile([128, F], BF16, name="wobv4_t")
            mask4_t = smalls.tile([128, 128], BF16, name="mask4_t")
            ones4_t = smalls.tile([128, 128], BF16, name="ones4_t")
            nc.vector.memset(ones4_t[:], 1.0)
            bo_t = smalls.tile([128, NFT], F32, name="bo_t")
            onec_t = smalls.tile([128, 1], BF16, name="onec_t")
            nc.vector.memset(onec_t[:], 1.0)
            stageP = smalls.tile([128, SH], BF16, name="stageP")
            nc.vector.memset(stageP[:], 0.0)
            P4_sb = smalls.tile([128, SH], BF16, name="P4_sb")
            dacc = smalls.tile([128, 2 * NKT], F32, name="dacc")
            den = smalls.tile([128, NKT], F32, name="den")
            deng = smalls.tile([128, NKT], F32, name="deng")
            inv = smalls.tile([128, NKT], F32, name="inv")
            # ScalarE exp-table warm-up: the first ACTIVATE pays the
            # ~2.7us ACT_TABLE_LOAD; issuing a dummy exp at t=0 hides it
            # under the DMA ramp instead of the first score chunk.
            warm_t = smalls.tile([1, 8], F32, name="warm_t")
            warm2_t = smalls.tile([1, 8], F32, name="warm2_t")
            nc.vector.memset(warm_t[:], 0.0)
            nc.scalar.activation(
                warm2_t[:], warm_t[:], mybir.ActivationFunctionType.Exp
            )
            # Collective-firmware warm-up: the first AllReduce of a NEFF
            # pays ~11us ncfw trigger delay + ~18us first-call duration
            # (measured); issuing a tiny dummy AllReduce at t~0 absorbs
            # both while the PE is busy with the qk chains, so the real
            # denominator AllReduces run at their warm ~6us latency.
            warm_cc_in = dram.tile([1, 8], F32, name="warm_cc_in")
            warm_cc_out = dram.tile([1, 8], F32, name="warm_cc_out")
            nc.gpsimd.dma_start(warm_cc_in[:], warm_t[:])
            nc.gpsimd.collective_compute(
                "AllReduce",
                mybir.AluOpType.add,
                replica_groups=REPLICA_GROUPS,
                ins=[warm_cc_in.opt()],
                outs=[warm_cc_out.opt()],
            )
            qkT = [qkv.tile([128, SH], BF16, name=f"qkT{i}") for i in range(NFT)]
            xk_t = [qkv.tile([128, S], BF16, name=f"xk{i}") for i in range(NFT)]
            xkN_t = [qkv.tile([128, F], BF16, name=f"xkN{i}") for i in range(NKT)]

            with tc.tile_pool(name="ph1", bufs=1) as ph1:
                xq_t = [ph1.tile([128, SH], BF16, name=f"xq{i}") for i in range(NFT)]
                wk_t = [ph1.tile([128, F], BF16, name=f"wk{i}") for i in range(NFT)]
                # DMAs in consumption order: Q operands (split in two f
                # halves to let the PE start after only 2MB has landed),
                # then K operands, then V weights.
                for half in range(2):
                    for i in range(half * NFT // 2, (half + 1) * NFT // 2):
                        r = slice(i * 128, (i + 1) * 128)
                        nc.sync.dma_start(wk_t[i][:], wqk[r, :])
                        nc.sync.dma_start(xq_t[i][:], xqT[r, :])
                for j in range(4):
                    nc.sync.dma_start(
                        cq4_t[32 * j : 32 * j + 1, :], cq4[j : j + 1, :]
                    )
                    nc.sync.dma_start(
                        wobv4_t[32 * j : 32 * j + 1, :], wobv4[j : j + 1, :]
                    )
                nc.sync.dma_start(mask4_t[:], mask4)
                nc.sync.dma_start(bo_t[:], bor)
                for i in range(NFT):
                    r = slice(i * 128, (i + 1) * 128)
                    nc.sync.dma_start(xk_t[i][:], xkT[r, :])
                for i in range(NKT):
                    r = slice(i * 128, (i + 1) * 128)
                    nc.sync.dma_start(xkN_t[i][:], xkN[r, :])

                # Fused Q/K: the host precomputes Wqk = Wq.T @ Wk, so
                # qk^T[f, q] = sum_f1 Wqk[f1, f] * xq^T[f1, q] directly from
                # the input activations (no Q or K projection on device).
                # Chains split into f1-halves (A: 0..3, B: 4..7) in groups
                # of 8 open PSUM accumulations so the A parts only need the
                # first half of the DMAs.
                qchains = [(fi, qb) for fi in range(NFT) for qb in range(NQB)]
                for grp in range(0, len(qchains), 8):
                    group = qchains[grp : grp + 8]
                    # First group starts after only 1 of 8 operand-tile DMA
                    # pairs (0.5MB) so the PE ramps in earlier.
                    asplit = 1 if grp == 0 else NFT // 2
                    qps = {}
                    for fi, qb in group:
                        fsl = slice(fi * 128, (fi + 1) * 128)
                        qsl = slice(qb * 512, (qb + 1) * 512)
                        ps = psum.tile([128, 512], F32, name="ps", tag="ps")
                        qps[(fi, qb)] = ps
                        for f1 in range(asplit):
                            nc.tensor.matmul(
                                ps[:], wk_t[f1][:, fsl], xq_t[f1][:, qsl],
                                start=(f1 == 0), stop=False,
                            )
                    for fi, qb in group:
                        fsl = slice(fi * 128, (fi + 1) * 128)
                        qsl = slice(qb * 512, (qb + 1) * 512)
                        ps = qps[(fi, qb)]
                        for f1 in range(asplit, NFT):
                            nc.tensor.matmul(
                                ps[:], wk_t[f1][:, fsl], xq_t[f1][:, qsl],
                                start=False, stop=(f1 == NFT - 1),
                            )
                        nc.vector.tensor_copy(qkT[fi][:, qsl], ps[:])

            with tc.tile_pool(name="ph2", bufs=1) as ph2:
                p_t = [ph2.tile([128, SH], BF16, name=f"p{i}") for i in range(NKT)]
                wov_t = [ph2.tile([128, F], BF16, name=f"wov{i}") for i in range(NFT)]
                xp_t = [ph2.tile([128, SH], BF16, name=f"xp{i}") for i in range(NFT)]
                for i in range(NFT):
                    nc.sync.dma_start(wov_t[i][:], wovT[i * 128 : (i + 1) * 128, :])

                # scores^T[k, q] -> exp(scale*.) -> p (bf16) + per-key rowsums.
                # The key axis is processed in 3 chunks of [8,4,4] k-tiles;
                # each chunk's denominator AllReduce is issued as soon as the
                # chunk's scores are done, so chunk 0's collective hides
                # under chunk 1's score matmuls and the last chunk's
                # collective hides under the ctx matmuls on earlier chunks.
                # The +cq restores that close each (ki, qb) chain are K=1
                # matmuls; they are deferred and issued 4-at-a-time on
                # disjoint PE row groups (tile_position) so 4 of them cost
                # ~512 cycles instead of 4*512.
                CH_BOUNDS = [0, 8, 16]   # k-tile chunk boundaries
                NCH = len(CH_BOUNDS) - 1  # k-tiles per chunk
                prev_readback = None
                cc_ins = [
                    dram.tile([128, CH_BOUNDS[c + 1] - CH_BOUNDS[c]], F32,
                              name=f"cc_in{c}")
                    for c in range(NCH)
                ]
                cc_outs = [
                    dram.tile([128, CH_BOUNDS[c + 1] - CH_BOUNDS[c]], F32,
                              name=f"cc_out{c}")
                    for c in range(NCH)
                ]
                for ch in range(NCH):
                    for ki0 in range(CH_BOUNDS[ch], CH_BOUNDS[ch + 1], 2):
                        # one restore-group = 2 k-tiles x 2 q-blocks
                        pss = {}
                        for ki in (ki0, ki0 + 1):
                            ksl = slice(ki * 128, (ki + 1) * 128)
                            for qb in range(NQB):
                                pss[(ki, qb)] = psum.tile(
                                    [128, 512], F32, name="ps", tag="ps"
                                )
                            # stationary-major: both q-block chains consume
                            # the same xk stationary tile back-to-back,
                            # halving the LDWEIGHTS traffic.
                            for fi in range(NFT):
                                for qb in range(NQB):
                                    qsl = slice(qb * 512, (qb + 1) * 512)
                                    nc.tensor.matmul(
                                        pss[(ki, qb)][:], xk_t[fi][:, ksl],
                                        qkT[fi][:, qsl],
                                        start=(fi == 0), stop=False,
                                    )
                        # 4 concurrent K=1 +cq restores on row groups
                        # 0/32/64/96 (cq staged at those partitions).
                        for ki in (ki0, ki0 + 1):
                            for qb in range(NQB):
                                j = (ki - ki0) * 2 + qb
                                qsl = slice(qb * 512, (qb + 1) * 512)
                                nc.tensor.matmul(
                                    pss[(ki, qb)][:],
                                    ones4_t[32 * j : 32 * j + 1, :],
                                    cq4_t[32 * j : 32 * j + 1, qsl],
                                    start=False, stop=True,
                                    tile_position=(32 * j, 0),
                                )
                        for ki in (ki0, ki0 + 1):
                            for qb in range(NQB):
                                qsl = slice(qb * 512, (qb + 1) * 512)
                                jj = qb * NKT + ki
                                nc.scalar.activation(
                                    p_t[ki][:, qsl], pss[(ki, qb)][:],
                                    mybir.ActivationFunctionType.Exp,
                                    scale=SCALE,
                                    accum_out=dacc[:, jj : jj + 1],
                                )
                    # local chunk denominators -> pair AllReduce -> 1/x
                    c0, c1 = CH_BOUNDS[ch], CH_BOUNDS[ch + 1]
                    csl = slice(c0, c1)
                    nc.vector.tensor_add(
                        den[:, csl], dacc[:, c0:c1], dacc[:, NKT + c0 : NKT + c1]
                    )
                    cin_dma = nc.gpsimd.dma_start(cc_ins[ch][:], den[:, csl])
                    if ch > 0 and prev_readback is not None:
                        # Keep the gpsimd stream in dataflow order: chunk
                        # ch's bounce write must not be scheduled ahead of
                        # chunk ch-1's result readback, else the readback
                        # (and the dependent attn scaling) stalls behind
                        # chunk ch's exp tail.
                        add_dep_helper(
                            cin_dma.ins, prev_readback.ins, False,
                            "AR bounce order: readback before next chunk in",
                        )
                    nc.gpsimd.collective_compute(
                        "AllReduce",
                        mybir.AluOpType.add,
                        replica_groups=REPLICA_GROUPS,
                        ins=[cc_ins[ch].opt()],
                        outs=[cc_outs[ch].opt()],
                    )
                    prev_readback = nc.gpsimd.dma_start(deng[:, csl], cc_outs[ch][:])
                    nc.vector.reciprocal(inv[:, csl], deng[:, csl])
                    # attn^T = p * inv[k]  (per-partition scalar, in place)
                    for ki in range(CH_BOUNDS[ch], CH_BOUNDS[ch + 1]):
                        nc.vector.tensor_scalar_mul(
                            p_t[ki][:], p_t[ki][:], inv[:, ki : ki + 1]
                        )

                # Associativity rewrite of the V side: instead of
                # projecting V for all 2048 keys and ctx = attn^T-contracted
                # V, compute xp[f, q] = sum_k x[k, f] attn[k, q] (per-query,
                # no duplicated work) and apply Wv afterwards:
                # ctx[d, q] = sum_f Wv[d, f] xp[f, q] + bv[d] * P[q] with
                # P[q] = sum_k attn[k, q]. Chains split by k-chunk (A:
                # tiles before the last AllReduce chunk, B: rest) in groups
                # of 8 open PSUM accumulations so the A parts execute while
                # the last denominator AllReduce is in flight.
                cchains = [(fi, qb) for fi in range(NFT) for qb in range(NQB)]
                for grp in range(0, len(cchains), 8):
                    group = cchains[grp : grp + 8]
                    cps = {
                        c: psum.tile([128, 512], F32, name="ps", tag="ps")
                        for c in group
                    }
                    # stationary-major: for each k-tile, the q-block pair of
                    # every fi chain reuses the same xkN stationary tile.
                    for ki in range(CH_BOUNDS[-2]):
                        for fi, qb in group:
                            fsl = slice(fi * 128, (fi + 1) * 128)
                            qsl = slice(qb * 512, (qb + 1) * 512)
                            nc.tensor.matmul(
                                cps[(fi, qb)][:], xkN_t[ki][:, fsl],
                                p_t[ki][:, qsl],
                                start=(ki == 0), stop=False,
                            )
                    for ki in range(CH_BOUNDS[-2], NKT):
                        for fi, qb in group:
                            fsl = slice(fi * 128, (fi + 1) * 128)
                            qsl = slice(qb * 512, (qb + 1) * 512)
                            nc.tensor.matmul(
                                cps[(fi, qb)][:], xkN_t[ki][:, fsl],
                                p_t[ki][:, qsl],
                                start=False, stop=(ki == NKT - 1),
                            )
                    for fi, qb in group:
                        fsl = slice(fi * 128, (fi + 1) * 128)
                        qsl = slice(qb * 512, (qb + 1) * 512)
                        nc.vector.tensor_copy(xp_t[fi][:, qsl], cps[(fi, qb)][:])

                # P[q] = sum_k attn[k, q]: 4 concurrent col-tiled M=1
                # chains per q-block (4 k-tiles each, own PSUM banks,
                # output partition 32j), then one mask-matmul per q-block
                # that sums the 4 partials and broadcasts P to partitions
                # {0,32,64,96} for the packed out-chain restores. The
                # combines are deferred until after the first out-group's
                # mains so the stage casts (VectorE) never stall the PE.
                psP = {}
                for qb in range(NQB):
                    qsl = slice(qb * 512, (qb + 1) * 512)
                    psP[qb] = [
                        psum.tile([128, 512], F32, name="psp", tag="ps")
                        for _ in range(4)
                    ]
                    for j in range(4):
                        for t in range(4):
                            ki = 4 * j + t
                            nc.tensor.matmul(
                                psP[qb][j][32 * j : 32 * j + 1, :],
                                onec_t[:, 0:1], p_t[ki][:, qsl],
                                start=(t == 0), stop=(t == 3),
                                tile_position=(0, 32 * j),
                            )
                    for j in range(4):
                        nc.vector.tensor_copy(
                            stageP[32 * j : 32 * j + 1, qsl],
                            psP[qb][j][32 * j : 32 * j + 1, :],
                        )

                # out^T[f', q] = sum_f (Wo@Wv)[f', f] xp[f, q]
                #                + (Wo@bv)[f'] P[q] + bo[f']
                # Chains in groups of 4 so the K=1 (Wo@bv)*P restores can
                # close 4 chains concurrently on disjoint PE row groups.
                ochains = [(fi, qb) for fi in range(NFT) for qb in range(NQB)]
                for grp in range(0, len(ochains), 4):
                    group = ochains[grp : grp + 4]
                    ops = {}
                    for fi, qb in group:
                        fsl = slice(fi * 128, (fi + 1) * 128)
                        qsl = slice(qb * 512, (qb + 1) * 512)
                        ps = psum.tile([128, 512], F32, name="ps", tag="ps")
                        ops[(fi, qb)] = ps
                        for fj in range(NFT):
                            nc.tensor.matmul(
                                ps[:], wov_t[fj][:, fsl], xp_t[fj][:, qsl],
                                start=(fj == 0), stop=False,
                            )
                    if grp == 0:
                        # P-partial combine + broadcast, issued once the
                        # stage casts have had a whole out-group of PE time
                        # to drain on VectorE.
                        for qb in range(NQB):
                            qsl = slice(qb * 512, (qb + 1) * 512)
                            psC = psum.tile([128, 512], F32, name="psc", tag="ps")
                            nc.tensor.matmul(
                                psC[:], mask4_t[:], stageP[:, qsl],
                                start=True, stop=True,
                            )
                            nc.vector.tensor_copy(P4_sb[0:97, qsl], psC[0:97, :])
                    for j, (fi, qb) in enumerate(group):
                        fsl = slice(fi * 128, (fi + 1) * 128)
                        qsl = slice(qb * 512, (qb + 1) * 512)
                        nc.tensor.matmul(
                            ops[(fi, qb)][:],
                            wobv4_t[32 * j : 32 * j + 1, fsl],
                            P4_sb[32 * j : 32 * j + 1, qsl],
                            start=False, stop=True,
                            tile_position=(32 * j, 0),
                        )
                    for fi, qb in group:
                        fsl = slice(fi * 128, (fi + 1) * 128)
                        qsl = slice(qb * 512, (qb + 1) * 512)
                        ot = ph2.tile([128, 512], F32, name="ost", tag="ost", bufs=3)
                        nc.vector.tensor_scalar_add(
                            ot[:], ops[(fi, qb)][:], bo_t[:, fi : fi + 1]
                        )
                        nc.sync.dma_start(outT[fsl, qsl], ot[:])

    nc.compile()
    return nc


def _get_compiled():
    global _COMPILED
    if _COMPILED is None:
        _COMPILED = _build()
    return _COMPILED


def kernel(x, Wq, bq, Wk, bk, Wv, bv, Wo, bo):
    global LAST_RESULTS
    nc = _get_compiled()

    x = np.asarray(x, dtype=np.float32)
    Wqf = np.asarray(Wq, np.float32)
    Wkf = np.asarray(Wk, np.float32)
    Wvf = np.asarray(Wv, np.float32)
    Wof = np.asarray(Wo, np.float32)
    wqk = np.ascontiguousarray(Wqf.T @ Wkf).astype(BF)
    wovT = np.ascontiguousarray((Wof @ Wvf).T).astype(BF)
    wqbk = Wqf.T @ np.asarray(bk, np.float32)          # [F]
    wobv = Wof @ np.asarray(bv, np.float32)            # [F]
    wobv4 = np.ascontiguousarray(
        np.broadcast_to(wobv[None, :], (4, F))
    ).astype(BF)
    bor = np.ascontiguousarray(np.asarray(bo, np.float32).reshape(NFT, 128).T)
    m4 = np.zeros((128, 128), np.float32)
    for j in range(4):
        for m in range(4):
            m4[32 * j, 32 * m] = 1.0
    mask4 = m4.astype(BF)

    shared = {
        "wqk": wqk, "wovT": wovT, "wobv4": wobv4, "mask4": mask4, "bor": bor,
    }
    xkT_b = [np.ascontiguousarray(x[b].T).astype(BF) for b in range(B)]
    xkN_b = [np.ascontiguousarray(x[b]).astype(BF) for b in range(B)]
    cq_b = [x[b] @ wqbk for b in range(B)]             # [S] per batch
    in_maps = []
    for c in range(N_CORES):
        b, h = c // 2, c % 2
        xqT_c = np.ascontiguousarray(x[b, h * SH : (h + 1) * SH, :].T).astype(BF)
        cq_c = np.ascontiguousarray(
            np.broadcast_to(cq_b[b][None, h * SH : (h + 1) * SH], (4, SH))
        ).astype(BF)
        in_maps.append(
            {"xqT": xqT_c, "xkT": xkT_b[b], "xkN": xkN_b[b], "cq4": cq_c,
             **shared}
        )

    res = run_bass_kernel_spmd(nc, in_maps, list(range(N_CORES)))
    LAST_RESULTS = res

    out = np.empty((B, S, F), np.float32)
    for c in range(N_CORES):
        b, h = c // 2, c % 2
        out[b, h * SH : (h + 1) * SH, :] = res.results[c]["outT"].T
    return out
